# revision 9
# baseline (speedup 1.0000x reference)
"""GraphSAGE (2x SAGEConv + linear) on 8 Trainium2 NeuronCores.

Sharding: nodes split into 8 contiguous ranges (dst-partitioned); edges live
with their destination core, grouped into 128-node dst blocks.  Per-edge
source features are fetched with dma_gather; mean aggregation is a one-hot
matmul on the TensorEngine (one-hot = is_equal(iota, lidx) * deg_inv, fused in
one DVE tensor_scalar).  Layer outputs are computed in [hid, node]
orientation so the SAGE bias is a per-partition ACT bias.  The halo exchange
before layer 1 is an AllGather of h into a shared DRAM buffer.  The kernel is
SPMD: per-(block, half) edge groups are padded to the max tile count across
the 8 cores so all cores share one program.
"""

import sys

sys.path.insert(0, "/opt/trn_rl_repo")

import numpy as np

import concourse.bacc as bacc
import concourse.bass as bass
import concourse.mybir as mybir
from concourse.bass_utils import run_bass_kernel_spmd
from concourse.library_config import mlp as _mlp_lib
from concourse.tile import TileContext

C = 8          # cores
BLK = 128      # dst nodes per block
SB = 4         # blocks per superblock (gather batch granularity)
PAD_LIDX = 999.0


def _preprocess(x, edge_index, n_cores=C):
    N = x.shape[0]
    E = edge_index.shape[1]
    NPC = N // n_cores
    HALF = N // 2
    NBLK = (NPC + BLK - 1) // BLK
    NSB = (NBLK + SB - 1) // SB

    src = edge_index[0].astype(np.int64)
    dst = edge_index[1].astype(np.int64)

    deg = np.bincount(dst, minlength=N)
    deg_inv = (1.0 / np.maximum(deg, 1)).astype(np.float32)

    core = dst // NPC
    local = dst % NPC
    blk = local // BLK
    lidx = (local % BLK).astype(np.float32)
    half = (src >= HALF).astype(np.int64)
    srcrel = (src - half * HALF).astype(np.int64)

    # per (core, blk, half) edge counts -> uniform tile counts across cores
    gid = (core * NBLK + blk) * 2 + half
    cnt = np.bincount(gid, minlength=n_cores * NBLK * 2).reshape(n_cores, NBLK, 2)
    Tc = -(-cnt // BLK)              # ceil tiles per (core, blk, half)
    T = Tc.max(axis=0)               # (NBLK, 2) uniform across cores
    T[:, 0] = np.maximum(T[:, 0], 1)

    # stream order per core: sb asc; within sb: half in (0,1); within half: blk asc
    # group stream rank:
    sb_of_blk = np.arange(NBLK) // SB
    # enumerate groups in stream order and compute tile offsets
    group_tile_off = np.zeros((NBLK, 2), np.int64)
    batches = []   # per (sb, half): dict(tile_start, ntiles)
    tiles_blk = []  # per tile: block id
    tiles_flag = []  # per tile: (start, stop)
    tiles_slot = []  # per tile: slot within its gather batch
    tiles_half = []  # per tile: which half / gather batch
    t_cursor = 0
    for sbi in range(NSB):
        blks = range(sbi * SB, min((sbi + 1) * SB, NBLK))
        for h in (0, 1):
            b_start = t_cursor
            for b in blks:
                group_tile_off[b, h] = t_cursor
                t_cursor += T[b, h]
            batches.append(dict(sb=sbi, half=h, tile_start=b_start,
                                ntiles=t_cursor - b_start))
        # tiles in stream order: h-major, then block
        for h in (0, 1):
            for b in blks:
                nt0, nt1 = T[b, 0], T[b, 1]
                for k in range(T[b, h]):
                    tiles_blk.append(b)
                    start = (h == 0 and k == 0) if nt0 > 0 else (h == 1 and k == 0)
                    stop = (h == 1 and k == nt1 - 1) if nt1 > 0 else (h == 0 and k == nt0 - 1)
                    tiles_flag.append((start, stop))
                    tiles_slot.append(group_tile_off[b, h]
                                      - batches[2 * sbi + h]["tile_start"] + k)
                    tiles_half.append(h)
    T_total = t_cursor
    E_pad = T_total * BLK

    # per-core data arrays
    # stream position of each edge: group offset + rank within group
    sgi = (sb_of_blk[blk] * 2 + half) * NBLK + blk      # unique per (blk, half), stream-ordered
    order = np.argsort(core * (2 * NBLK * NBLK + 10) + sgi, kind="stable")
    so_core = core[order]
    so_gid = gid[order]
    # rank within each (core, blk, half) run
    run_start = np.zeros(E, np.int64)
    new_run = np.ones(E, bool)
    new_run[1:] = so_gid[1:] != so_gid[:-1]
    run_idx = np.flatnonzero(new_run)
    run_start[run_idx] = run_idx
    run_start = np.maximum.accumulate(run_start)
    rank = np.arange(E) - run_start

    pos = group_tile_off[blk[order], half[order]] * BLK + rank

    idx_arr = np.zeros((n_cores, E_pad), np.int16)
    lid_arr = np.full((n_cores, E_pad), PAD_LIDX, np.float32)
    deg_arr = np.zeros((n_cores, E_pad), np.float32)
    ev_src = srcrel[order]
    ev_lid = lidx[order]
    ev_deg = deg_inv[dst[order]]
    for c in range(n_cores):
        m = so_core == c
        idx_arr[c, pos[m]] = ev_src[m].astype(np.int16)
        lid_arr[c, pos[m]] = ev_lid[m]
        deg_arr[c, pos[m]] = ev_deg[m]

    # device layouts
    idx16 = np.ascontiguousarray(
        np.tile(idx_arr.reshape(n_cores, -1, 16).transpose(0, 2, 1), (1, 8, 1)))
    lidxf = np.ascontiguousarray(lid_arr.reshape(n_cores, T_total, BLK).transpose(0, 2, 1))
    degf = np.ascontiguousarray(deg_arr.reshape(n_cores, T_total, BLK).transpose(0, 2, 1))

    return dict(N=N, E=E, NPC=NPC, HALF=HALF, NBLK=NBLK, NSB=NSB,
                T=T, T_total=T_total, batches=batches,
                tiles_blk=tiles_blk, tiles_flag=tiles_flag, tiles_slot=tiles_slot,
                tiles_halfarr=tiles_half,
                idx16=idx16, lidxf=lidxf, degf=degf)


def _build_program(meta, IN, HID, OUT):
    N, NPC, HALF = meta["N"], meta["NPC"], meta["HALF"]
    NBLK, NSB, T = meta["NBLK"], meta["NSB"], meta["T"]
    T_total = meta["T_total"]
    batches = meta["batches"]
    tiles_blk = meta["tiles_blk"]
    tiles_flag = meta["tiles_flag"]
    tiles_slot = meta["tiles_slot"]
    NPAD = NBLK * BLK
    f32 = mybir.dt.float32
    max_batch = max(b["ntiles"] for b in batches)

    nc = bacc.Bacc("TRN2", num_devices=C)

    x = nc.dram_tensor("x", [N, IN], f32, kind="ExternalInput")
    xT = nc.dram_tensor("xT", [IN, NPAD], f32, kind="ExternalInput")
    idx16 = nc.dram_tensor("idx16", [128, T_total * 8], mybir.dt.int16, kind="ExternalInput")
    lidxf = nc.dram_tensor("lidxf", [128, T_total], f32, kind="ExternalInput")
    degf = nc.dram_tensor("degf", [128, T_total], f32, kind="ExternalInput")
    iota = nc.dram_tensor("iota", [128, BLK], f32, kind="ExternalInput")
    ident = nc.dram_tensor("ident", [128, 128], f32, kind="ExternalInput")
    Wl0 = nc.dram_tensor("Wl0", [IN, HID], f32, kind="ExternalInput")
    Wr0 = nc.dram_tensor("Wr0", [IN, HID], f32, kind="ExternalInput")
    Wl1 = nc.dram_tensor("Wl1", [HID, HID], f32, kind="ExternalInput")
    Wr1 = nc.dram_tensor("Wr1", [HID, HID], f32, kind="ExternalInput")
    Wfc = nc.dram_tensor("Wfc", [HID, OUT], f32, kind="ExternalInput")
    bl0 = nc.dram_tensor("bl0", [HID, 1], f32, kind="ExternalInput")
    bl1 = nc.dram_tensor("bl1", [HID, 1], f32, kind="ExternalInput")
    bfc = nc.dram_tensor("bfc", [OUT, 1], f32, kind="ExternalInput")
    outT = nc.dram_tensor("outT", [OUT, NPC], f32, kind="ExternalOutput")

    h_loc = nc.dram_tensor("h_loc", [NPC, HID], f32, kind="Internal")
    h_full = nc.dram_tensor("h_full", [N, HID], f32, kind="Internal", addr_space="Shared")

    nc.gpsimd.load_library(_mlp_lib)

    with TileContext(nc) as tc:
        with (
            tc.tile_pool(name="cpool", bufs=1) as cp,
            tc.tile_pool(name="gpool", bufs=4) as gp,
            tc.tile_pool(name="ohpool", bufs=6) as ohp,
            tc.tile_pool(name="aggpool", bufs=2 * SB) as aggp,
            tc.tile_pool(name="smallpool", bufs=4) as smp,
            tc.tile_pool(name="sbpsum", bufs=SB, space="PSUM") as sbps,
            tc.tile_pool(name="hpsum", bufs=2, space="PSUM") as hps,
            tc.tile_pool(name="trpsum", bufs=1, space="PSUM") as trps,
            tc.tile_pool(name="opsum", bufs=1, space="PSUM") as ops,
        ):
            def cload(name, dram, shape, dtype=f32):
                t = cp.tile(shape, dtype, tag=name)
                nc.sync.dma_start(out=t[:], in_=dram[:])
                return t

            t_iota = cload("iota", iota, [128, BLK])
            t_id = cload("ident", ident, [128, 128])
            t_wl0 = cload("Wl0", Wl0, [IN, HID])
            t_wr0 = cload("Wr0", Wr0, [IN, HID])
            t_wl1 = cload("Wl1", Wl1, [HID, HID])
            t_wr1 = cload("Wr1", Wr1, [HID, HID])
            t_wfc = cload("Wfc", Wfc, [HID, OUT])
            t_bl0 = cload("bl0", bl0, [HID, 1])
            t_bl1 = cload("bl1", bl1, [HID, 1])
            t_bfc = cload("bfc", bfc, [OUT, 1])
            t_idx = cload("idx16", idx16, [128, T_total * 8], mybir.dt.int16)
            t_lid = cload("lidxf", lidxf, [128, T_total])
            t_deg = cload("degf", degf, [128, T_total])
            t_xT = cload("xT", xT, [IN, NPAD])
            t_hT = cp.tile([HID, NPAD], f32, tag="hT")

            for layer in (0, 1):
                F = IN if layer == 0 else HID
                src_lo = (x if layer == 0 else h_full)[0:HALF, :]
                src_hi = (x if layer == 0 else h_full)[HALF:N, :]
                ti = 0
                for sbi in range(NSB):
                    blks = list(range(sbi * SB, min((sbi + 1) * SB, NBLK)))
                    # gather both halves of this superblock
                    gtiles = []
                    for h in (0, 1):
                        bat = batches[sbi * 2 + h]
                        nt = bat["ntiles"]
                        g = gp.tile([128, max_batch, F], f32, tag="g")
                        if nt > 0:
                            K = nt * BLK
                            nc.gpsimd.dma_gather(
                                g[:, :nt, :],
                                src_lo if h == 0 else src_hi,
                                t_idx[:, bat["tile_start"] * 8: (bat["tile_start"] + nt) * 8],
                                K, K, F, single_packet=False)
                        gtiles.append(g)
                    ps_blk = {}
                    for b in blks:
                        pt = sbps.tile([F, BLK], f32, tag="sbp")
                        ps_blk[b] = pt
                    ntile_sb = sum(T[b, 0] + T[b, 1] for b in blks)
                    for _ in range(ntile_sb):
                        b = tiles_blk[ti]
                        st, sp = tiles_flag[ti]
                        slot = tiles_slot[ti]
                        oh = ohp.tile([128, BLK], f32, tag="oh")
                        nc.vector.tensor_scalar(
                            out=oh[:], in0=t_iota[:],
                            scalar1=t_lid[:, ti:ti + 1], scalar2=t_deg[:, ti:ti + 1],
                            op0=mybir.AluOpType.is_equal, op1=mybir.AluOpType.mult)
                        nc.tensor.matmul(
                            out=ps_blk[b][:],
                            lhsT=gtiles[tiles_half(meta, ti)][:, slot, :],
                            rhs=oh[:], start=st, stop=sp)
                        ti += 1
                    for b in blks:
                        nb = min(BLK, NPC - b * BLK)
                        agg = aggp.tile([F, BLK], f32, tag="agg")
                        nc.vector.tensor_copy(out=agg[:], in_=ps_blk[b][:])
                        php = hps.tile([HID, BLK], f32, tag="php")
                        if layer == 0:
                            nc.tensor.matmul(out=php[:], lhsT=t_wl0[:], rhs=agg[:],
                                             start=True, stop=False)
                            nc.tensor.matmul(out=php[:], lhsT=t_wr0[:],
                                             rhs=t_xT[:, b * BLK:(b + 1) * BLK],
                                             start=False, stop=True)
                            nc.scalar.activation(
                                out=t_hT[:, b * BLK:(b + 1) * BLK], in_=php[:],
                                func=mybir.ActivationFunctionType.Relu,
                                bias=t_bl0[:, 0:1])
                            ptr = trps.tile([128, HID], f32, tag="ptr")
                            nc.tensor.transpose(out=ptr[:], in_=t_hT[:, b * BLK:(b + 1) * BLK],
                                                identity=t_id[:])
                            hrow = smp.tile([128, HID], f32, tag="hrow")
                            nc.vector.tensor_copy(out=hrow[:], in_=ptr[:])
                            nc.sync.dma_start(out=h_loc[b * BLK:b * BLK + nb, :],
                                              in_=hrow[:nb, :])
                        else:
                            nc.tensor.matmul(out=php[:], lhsT=t_wl1[:], rhs=agg[:],
                                             start=True, stop=False)
                            nc.tensor.matmul(out=php[:], lhsT=t_wr1[:],
                                             rhs=t_hT[:, b * BLK:(b + 1) * BLK],
                                             start=False, stop=True)
                            h2 = smp.tile([HID, BLK], f32, tag="h2")
                            nc.scalar.activation(
                                out=h2[:], in_=php[:],
                                func=mybir.ActivationFunctionType.Relu,
                                bias=t_bl1[:, 0:1])
                            po = ops.tile([OUT, BLK], f32, tag="po")
                            nc.tensor.matmul(out=po[:], lhsT=t_wfc[:], rhs=h2[:],
                                             start=True, stop=True)
                            ot = smp.tile([OUT, BLK], f32, tag="ot")
                            nc.scalar.activation(
                                out=ot[:], in_=po[:],
                                func=mybir.ActivationFunctionType.Identity,
                                bias=t_bfc[:, 0:1])
                            nc.sync.dma_start(out=outT[:, b * BLK:b * BLK + nb],
                                              in_=ot[:, :nb])
                if layer == 0:
                    nc.gpsimd.collective_compute(
                        "AllGather", mybir.AluOpType.bypass,
                        replica_groups=[list(range(C))],
                        ins=[h_loc[:]], outs=[h_full[:]])

    nc.compile()
    return nc


def tiles_half(meta, ti):
    """Which half (gather batch within the superblock) tile ti belongs to."""
    return meta["tiles_halfarr"][ti]


def _finish_meta(meta):
    return meta


_CACHE = {}


def kernel(x, edge_index, W_l0, b_l0, W_r0, W_l1, b_l1, W_r1, W_fc, b_fc):
    x = np.asarray(x, np.float32)
    edge_index = np.asarray(edge_index, np.int32)
    N, IN = x.shape
    HID = W_l0.shape[0]
    OUT = W_fc.shape[0]
    NPC = N // C

    key = hash(edge_index.tobytes())
    if key in _CACHE:
        meta, nc = _CACHE[key]
    else:
        meta = _finish_meta(_preprocess(x, edge_index))
        nc = _build_program(meta, IN, HID, OUT)
        _CACHE[key] = (meta, nc)

    NPAD = meta["NBLK"] * BLK
    xTfull = np.zeros((C, IN, NPAD), np.float32)
    for c in range(C):
        xTfull[c, :, :NPC] = x[c * NPC:(c + 1) * NPC].T

    common = dict(
        x=np.ascontiguousarray(x),
        iota=np.tile(np.arange(BLK, dtype=np.float32), (128, 1)),
        ident=np.eye(128, dtype=np.float32),
        Wl0=np.ascontiguousarray(np.asarray(W_l0, np.float32).T),
        Wr0=np.ascontiguousarray(np.asarray(W_r0, np.float32).T),
        Wl1=np.ascontiguousarray(np.asarray(W_l1, np.float32).T),
        Wr1=np.ascontiguousarray(np.asarray(W_r1, np.float32).T),
        Wfc=np.ascontiguousarray(np.asarray(W_fc, np.float32).T),
        bl0=np.asarray(b_l0, np.float32).reshape(-1, 1),
        bl1=np.asarray(b_l1, np.float32).reshape(-1, 1),
        bfc=np.asarray(b_fc, np.float32).reshape(-1, 1),
    )
    in_maps = []
    for c in range(C):
        m = dict(common)
        m["xT"] = np.ascontiguousarray(xTfull[c])
        m["idx16"] = np.ascontiguousarray(meta["idx16"][c])
        m["lidxf"] = np.ascontiguousarray(meta["lidxf"][c])
        m["degf"] = np.ascontiguousarray(meta["degf"][c])
        in_maps.append(m)

    res = run_bass_kernel_spmd(nc, in_maps, core_ids=list(range(C)))

    out = np.empty((N, OUT), np.float32)
    for c in range(C):
        out[c * NPC:(c + 1) * NPC, :] = res.results[c]["outT"].T
    return out
